# revision 4
# baseline (speedup 1.0000x reference)
"""Trainium2 Bass kernel for Atom2Bond GNN message passing (forward).

Computation: out[e, :] = relu(concat(atom[src_idx[e]], edge[e]) @ W + b)
  atom_embedding [10000, 128] f32, edge_embedding [640000, 64] f32,
  src_idx [640000] int, W [192, 128] f32, b [128] f32 -> out [640000, 128] f32

Strategy (8 NeuronCores, edges sharded 80000/core, padded to 81920):

  Host-side, per core, edges are SORTED by src_idx. For a 512-edge tile
  whose (sorted) source atoms span [lo, lo+K), the gathered atom matrix
  is piecewise constant in runs, so with the step matrix
      H[a, e] = 1 if e >= start_a else 0        (a = lo..lo+127, local)
  and the first-difference matrix dA[a] = atom[a] - atom[a-1] (dA[lo] =
  atom[lo]), the atom-side contribution telescopes:
      atom[src[e]] = sum_a dA[a] * H[a, e].
  Pre-multiplying by the atom half of W HOST-side, G_t = dA_tile @ Wa,
  the whole gather + atom matmul collapses to ONE on-device matmul:
      out_atom[o, e] = sum_a G_t[a, o] * H_t[a, e]  =  (G_t.T @ H_t)
  H_t is built on-chip in one DVE tensor_scalar(is_ge) op from a
  constant iota row and a per-tile per-partition "starts" vector.

  Per 512-edge tile: 1 DVE op (H), 2 PSUM-accumulating matmuls
  (atom part K=128, edge part K=64 against host-pre-transposed edge
  features), 1 fused bias+ReLU on the scalar engine (fp16 out).
  No gather, no on-chip transposes. fp16 everywhere on-chip (exact
  0/1 for H, ~2^-11 rounding for data), fp32 PSUM accumulation.
  Output is written transposed+sorted; host un-transposes + un-sorts.
"""

import numpy as np

FP16 = np.float16

N_NODES = 10000
N_EDGES = 640000
NODE_DIM = 128
EDGE_DIM = 64
N_CORES = 8

EPC = N_EDGES // N_CORES          # 80000 edges per core
TILE = 512                        # edges per matmul tile
CHUNK = 8192                      # edges per pipeline chunk (16 tiles)
TPC = CHUNK // TILE               # 16 tiles per chunk
EPAD = 81920                      # EPC padded to a multiple of CHUNK
NCHUNK = EPAD // CHUNK            # 10
NTILE = EPAD // TILE              # 160 tiles per core
PAD_IDX = N_NODES + 127           # pad edges point past real atoms (zeros)

TRACE = False                     # set True from test.py for NTFF profiling
LAST_RESULTS = None               # BassKernelResults of last run

_NC = None                        # cached compiled Bacc module


def _build_module():
    from contextlib import ExitStack

    import concourse.bacc as bacc
    import concourse.mybir as mybir
    import concourse.tile as tile

    nc = bacc.Bacc("TRN2", target_bir_lowering=False, debug=False)

    # Per-chunk-major host layouts so every chunk DMA is fully contiguous.
    gt = nc.dram_tensor(
        "gt", [NCHUNK, 128, TPC * 128], mybir.dt.float16, kind="ExternalInput"
    )
    starts = nc.dram_tensor(
        "starts", [NCHUNK, 128, TPC], mybir.dt.float32, kind="ExternalInput"
    )
    edget = nc.dram_tensor(
        "edget", [EDGE_DIM, EPAD], mybir.dt.float16, kind="ExternalInput"
    )
    we = nc.dram_tensor("we", [EDGE_DIM, 128], mybir.dt.float16, kind="ExternalInput")
    bias = nc.dram_tensor("bias", [128, 1], mybir.dt.float32, kind="ExternalInput")
    iota = nc.dram_tensor("iota", [128, TILE], mybir.dt.float16, kind="ExternalInput")
    outt = nc.dram_tensor("outt", [128, EPAD], mybir.dt.float16, kind="ExternalOutput")

    with tile.TileContext(nc) as tc, ExitStack() as ctx:
        singles = ctx.enter_context(tc.tile_pool(name="singles", bufs=1))
        gtp = ctx.enter_context(tc.tile_pool(name="gtp", bufs=2))
        stp = ctx.enter_context(tc.tile_pool(name="stp", bufs=2))
        edgep = ctx.enter_context(tc.tile_pool(name="edgep", bufs=2))
        outp = ctx.enter_context(tc.tile_pool(name="outp", bufs=2))
        hp = ctx.enter_context(tc.tile_pool(name="hp", bufs=4))
        psump = ctx.enter_context(tc.tile_pool(name="psum", bufs=8, space="PSUM"))

        iota_sb = singles.tile([128, TILE], mybir.dt.float16)
        nc.sync.dma_start(out=iota_sb[:], in_=iota[:])
        we_sb = singles.tile([EDGE_DIM, 128], mybir.dt.float16)
        nc.sync.dma_start(out=we_sb[:], in_=we[:])
        b_sb = singles.tile([128, 1], mybir.dt.float32)
        nc.sync.dma_start(out=b_sb[:], in_=bias[:])

        for c in range(NCHUNK):
            gt_sb = gtp.tile([128, TPC, 128], mybir.dt.float16)
            nc.sync.dma_start(
                out=gt_sb[:], in_=gt[c].rearrange("a (t f) -> a t f", t=TPC)
            )
            st_sb = stp.tile([128, TPC], mybir.dt.float32)
            nc.sync.dma_start(out=st_sb[:], in_=starts[c])
            edge_sb = edgep.tile([EDGE_DIM, CHUNK], mybir.dt.float16)
            nc.sync.dma_start(
                out=edge_sb[:], in_=edget[:, c * CHUNK : (c + 1) * CHUNK]
            )

            out_sb = outp.tile([128, CHUNK], mybir.dt.float16)
            for j in range(TPC):
                s = slice(j * TILE, (j + 1) * TILE)
                h_sb = hp.tile([128, TILE], mybir.dt.float16)
                nc.vector.tensor_scalar(
                    h_sb[:],
                    iota_sb[:],
                    st_sb[:, j : j + 1],
                    None,
                    mybir.AluOpType.is_ge,
                )
                ps = psump.tile([128, TILE], mybir.dt.float32)
                nc.tensor.matmul(
                    ps[:], gt_sb[:, j, :], h_sb[:], start=True, stop=False
                )
                nc.tensor.matmul(
                    ps[:], we_sb[:], edge_sb[:, s], start=False, stop=True
                )
                nc.scalar.activation(
                    out_sb[:, s],
                    ps[:],
                    mybir.ActivationFunctionType.Relu,
                    bias=b_sb[:],
                )
            nc.sync.dma_start(out=outt[:, c * CHUNK : (c + 1) * CHUNK], in_=out_sb[:])

    nc.compile()
    return nc


def _get_module():
    global _NC
    if _NC is None:
        _NC = _build_module()
    return _NC


def _install_axon_ntff_shim():
    """Register the NTFF profile hook that run_bass_kernel_spmd(trace=True)
    expects under axon; the agent image lacks antenv.axon_hooks."""
    import sys
    import types

    if "antenv.axon_hooks" in sys.modules:
        return
    try:
        from trn_agent_boot.trn_boot import _ntff_profile_via_ctypes

        hook = _ntff_profile_via_ctypes("/opt/axon/libaxon_pjrt.so")
    except Exception:
        hook = None
    mod = types.ModuleType("antenv.axon_hooks")
    mod.get_axon_ntff_profile_hook = lambda: hook
    mod.set_axon_ntff_profile_hook = lambda h: None
    sys.modules["antenv.axon_hooks"] = mod


def _prep_core_inputs(atom_embedding, edge_embedding, src_idx, W, b):
    """Host-side shard + sort + layout prep. Returns (in_maps, orders)."""
    atom_embedding = np.asarray(atom_embedding, dtype=np.float32)
    edge_embedding = np.asarray(edge_embedding, dtype=np.float32)
    src_idx = np.asarray(src_idx).astype(np.int64)
    W = np.asarray(W, dtype=np.float32)
    b = np.asarray(b, dtype=np.float32)

    # P[i] = atom_pad[i] @ Wa ; padded so any tile row slice is in range.
    n_pad = PAD_IDX + 1 + 128
    atom_pad = np.zeros((n_pad, NODE_DIM), np.float32)
    atom_pad[:N_NODES] = atom_embedding
    P = atom_pad @ W[:NODE_DIM]                    # [n_pad, 128] f32
    Pd = np.empty_like(P)                          # Pd[i] = P[i] - P[i-1]
    Pd[0] = P[0]
    Pd[1:] = P[1:] - P[:-1]

    we_h = np.ascontiguousarray(W[NODE_DIM:]).astype(FP16)
    b_h = np.ascontiguousarray(b.reshape(NODE_DIM, 1))
    iota_h = np.broadcast_to(
        np.arange(TILE, dtype=np.float32).astype(FP16), (128, TILE)
    ).copy()

    a128 = np.arange(128)
    in_maps = []
    orders = []
    for c in range(N_CORES):
        e0 = c * EPC
        idx_core = src_idx[e0 : e0 + EPC]
        order = np.argsort(idx_core, kind="stable")
        orders.append(order)
        sorted_idx = idx_core[order]
        # pad edges reuse the core's max atom id: keeps sort order and
        # keeps the last tile's atom span tight (outputs are discarded)
        sidx = np.full(EPAD, sorted_idx[-1], np.int64)
        sidx[:EPC] = sorted_idx

        tiles = sidx.reshape(NTILE, TILE)
        lo = tiles[:, 0]                            # [NTILE]
        span = tiles[:, -1] - lo
        assert span.max() <= 127, (
            f"tile atom span {span.max()} > 127; sorted-tile assumption broken"
        )

        # G[t, k] = P[lo_t + k] - P[lo_t + k - 1], with G[t, 0] = P[lo_t]
        rows = lo[:, None] + a128[None, :]          # [NTILE, 128]
        G = Pd[rows]                                # [NTILE, 128, 128] f32
        G[:, 0] = P[lo]
        # chunk-major, atom-partition-major layout: [NCHUNK, 128a, TPC, 128f]
        gt_h = np.ascontiguousarray(
            G.reshape(NCHUNK, TPC, 128, 128).transpose(0, 2, 1, 3)
        ).astype(FP16).reshape(NCHUNK, 128, TPC * 128)

        # starts[t, k] = first within-tile position with idx >= lo_t + k
        st = np.empty((NTILE, 128), np.int32)
        for t in range(NTILE):
            st[t] = np.searchsorted(tiles[t], lo[t] + a128, side="left")
        starts_h = np.ascontiguousarray(
            st.reshape(NCHUNK, TPC, 128).transpose(0, 2, 1)
        ).astype(np.float32)

        edget_h = np.zeros((EDGE_DIM, EPAD), FP16)
        edget_h[:, :EPC] = edge_embedding[e0 : e0 + EPC][order].T.astype(FP16)

        in_maps.append(
            {
                "gt": gt_h,
                "starts": starts_h,
                "edget": edget_h,
                "we": we_h,
                "bias": b_h,
                "iota": iota_h,
            }
        )
    return in_maps, orders


def kernel(atom_embedding, edge_embedding, src_idx, W, b):
    global LAST_RESULTS
    from concourse.bass_utils import run_bass_kernel_spmd

    nc = _get_module()
    in_maps, orders = _prep_core_inputs(
        atom_embedding, edge_embedding, src_idx, W, b
    )

    kwargs = {}
    if TRACE:
        _install_axon_ntff_shim()
        import concourse.bass_utils as bu

        bu.upload_artifacts = lambda tmpdir: tmpdir  # no bucket in this sandbox
        kwargs = dict(trace=True)

    res = run_bass_kernel_spmd(nc, in_maps, core_ids=list(range(N_CORES)), **kwargs)
    LAST_RESULTS = res

    out = np.empty((N_EDGES, NODE_DIM), np.float32)
    for c in range(N_CORES):
        outt = np.asarray(res.results[c]["outt"])   # [128, EPAD] fp16
        sorted_out = outt[:, :EPC].T.astype(np.float32)
        out[c * EPC + orders[c]] = sorted_out
    return out
